# revision 11
# baseline (speedup 1.0000x reference)
"""Trainium2 Bass kernel for AdjacencyLearning (gnn_message_passing).

Computes, for P=3 adjacency powers and E=600000 edges over 50000 nodes:
    w[p, e] = sigmoid(relu(|x[src] - x[tgt]| @ Wa[p] + ba[p]) @ Wb[p] + bb[p])

Strategy (8 NeuronCores, SPMD):
  - Shard the edge dimension E across the 8 cores (75000 edges/core/power).
  - Per edge, gather the two 128-feature node rows from an fp16 copy of x in
    HBM using the SWDGE dma_gather instruction in transpose mode, which lands
    tiles in [feat=128 partitions, edges free] layout - exactly the moving
    operand layout the TensorEngine needs. Gathers are 512 indices each (the
    SWDGE descriptor ring caps ~960 per instruction) spread over all 4 SWDGE
    queues; throughput is Q7 descriptor-generation bound (~2.8 ns/index).
  - dma_gather indices are int16, so node ids >= 32768 are unreachable
    directly. Host-side, edges are bucketed by (src>=32768, tgt>=32768) into
    4 contiguous slot ranges; each bucket's gathers read from a base-biased
    view of x and use rebased indices. The host inverse-permutes the output.
  - Per-edge MLP: TensorE matmul (fp16) -> PSUM, ScalarE relu+bias, TensorE
    block-diagonal second layer, ScalarE sigmoid+bias, DMA out.
    Four [3,512] layer-1 outputs are packed per PSUM bank at partition
    offsets 0/32/64/96 so activation/matmul instruction counts stay low.
"""

import numpy as np

import concourse.bass as bass
import concourse.mybir as mybir
import concourse.tile as tile
from concourse import bacc
from concourse.bass_utils import run_bass_kernel_spmd
from concourse.tile import add_dep_helper
from concourse.masks import make_identity

# Problem shape (hardcoded; kernel.py must be self-contained).
N_NODES = 50000
F = 128
P = 3
E = 600000
N_CORES = 8
E_PC = E // N_CORES          # 75000 edges per core per power
HALF = 32768                 # int16 gather index limit

TILE = 512                   # edges per gather / per layer-1 matmul
GROUP = 4                    # layer-1 outputs packed per PSUM bank (offsets 0/32/64/96)
NQ = 4                       # SWDGE queues
# Per-(core,power) bucket capacities in tiles for buckets (src_half, tgt_half)
# = (0,0),(0,1),(1,0),(1,1). Expected sizes 32210/16940/16940/8909 edges;
# caps leave >8 sigma of headroom. Sum must be divisible by GROUP.
CAPS_TILES = (66, 35, 35, 20)

F16 = mybir.dt.float16
F32 = mybir.dt.float32
I16 = mybir.dt.int16


def build(caps=CAPS_TILES, n_cores=N_CORES):
    """Build + compile the SPMD Bass program. Returns (nc, meta)."""
    tiles_pp = sum(caps)
    assert tiles_pp % GROUP == 0
    slots_pp = tiles_pp * TILE
    side = P * slots_pp
    idxcols = side // 16

    nc = bacc.Bacc("TRN2", target_bir_lowering=False, debug=False,
                   num_devices=n_cores, num_swdge_queues=NQ)
    x = nc.dram_tensor("xf16", [N_NODES, F], F16, kind="ExternalInput")
    idxj = nc.dram_tensor("idxj", [128, idxcols], I16, kind="ExternalInput")
    idxi = nc.dram_tensor("idxi", [128, idxcols], I16, kind="ExternalInput")
    wa = nc.dram_tensor("wa", [F, 3 * P], F16, kind="ExternalInput")
    wb = nc.dram_tensor("wb", [99, 4 * P], F16, kind="ExternalInput")
    bat = nc.dram_tensor("bat", [99, P], F32, kind="ExternalInput")
    bbt = nc.dram_tensor("bbt", [4, P], F32, kind="ExternalInput")
    out = nc.dram_tensor("out", [P, tiles_pp, TILE], F32, kind="ExternalOutput")

    # Static bucket tile ranges within a power.
    bstart = [0]
    for c in caps:
        bstart.append(bstart[-1] + c)

    with tile.TileContext(nc) as tc:
        with (
            tc.tile_pool(name="const", bufs=1) as constp,
            tc.tile_pool(name="gj", bufs=12) as gjp,
            tc.tile_pool(name="gi", bufs=12) as gip,
            tc.tile_pool(name="dt", bufs=4) as dtp,
            tc.tile_pool(name="hr", bufs=4) as hrp,
            tc.tile_pool(name="ost", bufs=4) as ostp,
            tc.tile_pool(name="hp", bufs=4, space="PSUM") as hpp,
            tc.tile_pool(name="tp", bufs=2, space="PSUM") as tpp,
            tc.tile_pool(name="wp", bufs=2, space="PSUM") as wpp,
        ):
            idxj_sb = constp.tile([128, idxcols], I16)
            nc.sync.dma_start(idxj_sb[:], idxj.ap())
            idxi_sb = constp.tile([128, idxcols], I16)
            nc.sync.dma_start(idxi_sb[:], idxi.ap())
            wa_sb = constp.tile([F, 3 * P], F16)
            nc.sync.dma_start(wa_sb[:], wa.ap())
            wb_sb = constp.tile([99, 4 * P], F16)
            nc.sync.dma_start(wb_sb[:], wb.ap())
            bat_sb = constp.tile([99, P], F32)
            nc.sync.dma_start(bat_sb[:], bat.ap())
            bbt_sb = constp.tile([4, P], F32)
            nc.sync.dma_start(bbt_sb[:], bbt.ap())
            ident = constp.tile([128, 128], F16)
            make_identity(nc, ident[:])

            # Layer-1 PSUM banks: zero once so the partitions the matmuls
            # never write stay 0 (the block-diagonal layer-2 weights hit them
            # with 0s; 0*0 must not be NaN*0).
            hp_tiles = []
            for _ in range(4):
                t = hpp.tile([128, TILE], F32, tag="hp")
                nc.vector.memset(t[:], 0.0)
                hp_tiles.append(t)

            x_full = x.ap()
            x_high = x.ap()[HALF:, :]

            qn = 0
            prev_gather = None
            for p in range(P):
                wa_ap = wa_sb[:, 3 * p:3 * p + 3]
                wb_ap = wb_sb[:, 4 * p:4 * p + 4]
                ba_ap = bat_sb[:, p:p + 1]
                bb_ap = bbt_sb[:, p:p + 1]
                for t in range(tiles_pp):
                    b = next(bi for bi in range(4) if bstart[bi] <= t < bstart[bi + 1])
                    hj, hi = b >> 1, b & 1
                    col0 = (p * slots_pp + t * TILE) // 16
                    # Non-transpose gather: edge k of the tile lands on
                    # partition k%128, block k//128. (Concurrent TRANSPOSE
                    # gathers on different SWDGE queues corrupt data - shared
                    # xbar state - so we gather untransposed and transpose on
                    # the TensorEngine instead.)
                    gj = gjp.tile([128, TILE // 128, F], F16, tag="gj")
                    gi = gip.tile([128, TILE // 128, F], F16, tag="gi")
                    gj_inst = nc.gpsimd.dma_gather(
                        gj[:, :, :], x_high if hj else x_full,
                        idxj_sb[:, col0:col0 + TILE // 16],
                        num_idxs=TILE, num_idxs_reg=TILE, elem_size=F,
                        transpose=False, queue_num=qn % NQ)
                    qn += 1
                    # Chain gathers in emission order (no sem): keeps the Pool
                    # program order equal to emission order so Tile's DMASW
                    # sem-lane round-robin (8 lanes) stays in lockstep with
                    # the queue cycle (4 queues) - each sem lane then only
                    # ever serves one SWDGE queue, which the runtime requires.
                    if prev_gather is not None:
                        add_dep_helper(gj_inst.ins, prev_gather.ins,
                                       sync=False, reason="swdge lane lockstep")
                    gi_inst = nc.gpsimd.dma_gather(
                        gi[:, :, :], x_high if hi else x_full,
                        idxi_sb[:, col0:col0 + TILE // 16],
                        num_idxs=TILE, num_idxs_reg=TILE, elem_size=F,
                        transpose=False, queue_num=qn % NQ)
                    qn += 1
                    add_dep_helper(gi_inst.ins, gj_inst.ins,
                                   sync=False, reason="swdge lane lockstep")
                    prev_gather = gi_inst
                    dj = gj[:, :, :].rearrange("p a b -> p (a b)")
                    nc.vector.tensor_tensor(
                        dj, dj, gi[:, :, :].rearrange("p a b -> p (a b)"),
                        mybir.AluOpType.subtract)
                    # fp16 |d| = clear the sign bit (abs_max isn't a valid
                    # HW tensor_scalar ALU op).
                    dj_i = dj.bitcast(I16)
                    nc.vector.tensor_scalar(dj_i, dj_i, 0x7FFF, None,
                                            mybir.AluOpType.bitwise_and)
                    # Transpose |d| blocks [128e,128f] -> [128f,128e] on PE,
                    # then PSUM -> SBUF copies split across ACT and DVE.
                    tp = tpp.tile([128, TILE // 128, 128], F16, tag="tp")
                    for blk in range(TILE // 128):
                        nc.tensor.transpose(tp[:, blk, :], gj[:, blk, :],
                                            ident[:])
                    dT = dtp.tile([128, TILE], F16, tag="dT")
                    tpf = tp[:, :, :].rearrange("p a b -> p (a b)")
                    half = TILE // 2
                    nc.scalar.activation(dT[:, :half], tpf[:, :half],
                                         mybir.ActivationFunctionType.Copy)
                    nc.vector.tensor_copy(dT[:, half:], tpf[:, half:])
                    g, q = divmod(t, GROUP)
                    hp = hp_tiles[g % 4]
                    nc.tensor.matmul(
                        hp[32 * q:32 * q + 3, :], lhsT=wa_ap, rhs=dT[:, :],
                        start=True, stop=True, tile_position=(0, 32 * q))
                    if q == GROUP - 1:
                        hr = hrp.tile([99, TILE], F16, tag="hr")
                        nc.scalar.activation(
                            hr[:], hp[:99, :],
                            mybir.ActivationFunctionType.Relu,
                            bias=ba_ap, scale=1.0)
                        wpt = wpp.tile([4, TILE], F32, tag="wp")
                        nc.tensor.matmul(wpt[:], lhsT=wb_ap, rhs=hr[:],
                                         start=True, stop=True)
                        ost = ostp.tile([4, TILE], F32, tag="ost")
                        nc.scalar.activation(
                            ost[:], wpt[:],
                            mybir.ActivationFunctionType.Sigmoid,
                            bias=bb_ap, scale=1.0)
                        nc.sync.dma_start(
                            out.ap()[p, GROUP * g:GROUP * g + GROUP, :],
                            ost[:])

    nc.compile()
    meta = {"tiles_pp": tiles_pp, "slots_pp": slots_pp, "caps": caps}
    return nc, meta


def prepare(inputs, caps=CAPS_TILES, n_cores=N_CORES, e_pc=E_PC):
    """Host-side sharding/bucketing. Returns (in_maps, slot_maps)."""
    x = np.asarray(inputs["x"])
    ei = np.asarray(inputs["edge_index"]).astype(np.int64)
    Wa = np.asarray(inputs["Wa"], dtype=np.float32)
    ba = np.asarray(inputs["ba"], dtype=np.float32)
    Wb = np.asarray(inputs["Wb"], dtype=np.float32)
    bb = np.asarray(inputs["bb"], dtype=np.float32)

    tiles_pp = sum(caps)
    slots_pp = tiles_pp * TILE
    caps_slots = np.asarray(caps, dtype=np.int64) * TILE
    starts = np.concatenate([[0], np.cumsum(caps_slots)[:-1]])

    xf16 = np.ascontiguousarray(x.astype(np.float16))
    wa_h = np.zeros((F, 3 * P), np.float16)
    wb_h = np.zeros((99, 4 * P), np.float16)
    bat_h = np.zeros((99, P), np.float32)
    bbt_h = np.zeros((4, P), np.float32)
    for p in range(P):
        wa_h[:, 3 * p:3 * p + 3] = Wa[p]
        bbt_h[:, p] = bb[p, 0]
        for q in range(GROUP):
            for k in range(3):
                wb_h[32 * q + k, 4 * p + q] = Wb[p, k, 0]
                bat_h[32 * q + k, p] = ba[p, k]

    def wrap(a):
        return np.tile(np.ascontiguousarray(a.reshape(-1, 16).T), (8, 1))

    in_maps, slot_maps = [], []
    for c in range(n_cores):
        idxj_all = np.zeros(P * slots_pp, np.int16)
        idxi_all = np.zeros(P * slots_pp, np.int16)
        slot_map = np.zeros((P, e_pc), np.int64)
        for p in range(P):
            j = ei[p, 0, c * e_pc:(c + 1) * e_pc]
            i = ei[p, 1, c * e_pc:(c + 1) * e_pc]
            hj = (j >= HALF).astype(np.int64)
            hi = (i >= HALF).astype(np.int64)
            bkt = hj * 2 + hi
            counts = np.bincount(bkt, minlength=4)
            if np.any(counts > caps_slots):
                raise RuntimeError(f"bucket overflow: {counts} vs {caps_slots}")
            order = np.argsort(bkt, kind="stable")
            slot_sorted = np.concatenate(
                [starts[b] + np.arange(counts[b]) for b in range(4)])
            slot = np.empty(e_pc, np.int64)
            slot[order] = slot_sorted
            base = p * slots_pp
            idxj_all[base + slot] = (j - HALF * hj).astype(np.int16)
            idxi_all[base + slot] = (i - HALF * hi).astype(np.int16)
            slot_map[p] = slot
        in_maps.append({
            "xf16": xf16,
            "idxj": wrap(idxj_all),
            "idxi": wrap(idxi_all),
            "wa": wa_h, "wb": wb_h, "bat": bat_h, "bbt": bbt_h,
        })
        slot_maps.append(slot_map)
    return in_maps, slot_maps


_CACHE = {}


def get_program():
    if "nc" not in _CACHE:
        _CACHE["nc"], _CACHE["meta"] = build()
    return _CACHE["nc"], _CACHE["meta"]


def postprocess(results, slot_maps, slots_pp, e_pc=E_PC, n_cores=N_CORES):
    out = np.empty((P, E), np.float32)
    for c in range(n_cores):
        w = results[c]["out"].reshape(P, slots_pp)
        out[:, c * e_pc:(c + 1) * e_pc] = np.take_along_axis(
            w, slot_maps[c], axis=1)
    return out


def kernel(**inputs) -> np.ndarray:
    nc, meta = get_program()
    in_maps, slot_maps = prepare(inputs)
    res = run_bass_kernel_spmd(nc, in_maps, core_ids=list(range(N_CORES)))
    return postprocess(res.results, slot_maps, meta["slots_pp"])


# revision 12
# speedup vs baseline: 1.0654x; 1.0654x over previous
"""Trainium2 Bass kernel for AdjacencyLearning (gnn_message_passing).

Computes, for P=3 adjacency powers and E=600000 edges over 50000 nodes:
    w[p, e] = sigmoid(relu(|x[src] - x[tgt]| @ Wa[p] + ba[p]) @ Wb[p] + bb[p])

Strategy (8 NeuronCores, SPMD):
  - Shard the edge dimension E across the 8 cores (75000 edges/core/power).
  - Per edge, gather the two 128-feature node rows from an fp16 copy of x in
    HBM using the SWDGE dma_gather instruction in transpose mode, which lands
    tiles in [feat=128 partitions, edges free] layout - exactly the moving
    operand layout the TensorEngine needs. Gathers are 512 indices each (the
    SWDGE descriptor ring caps ~960 per instruction) spread over all 4 SWDGE
    queues; throughput is Q7 descriptor-generation bound (~2.8 ns/index).
  - dma_gather indices are int16, so node ids >= 32768 are unreachable
    directly. Host-side, edges are bucketed by (src>=32768, tgt>=32768) into
    4 contiguous slot ranges; each bucket's gathers read from a base-biased
    view of x and use rebased indices. The host inverse-permutes the output.
  - Per-edge MLP: TensorE matmul (fp16) -> PSUM, ScalarE relu+bias, TensorE
    block-diagonal second layer, ScalarE sigmoid+bias, DMA out.
    Four [3,512] layer-1 outputs are packed per PSUM bank at partition
    offsets 0/32/64/96 so activation/matmul instruction counts stay low.
"""

import numpy as np

import concourse.bass as bass
import concourse.mybir as mybir
import concourse.tile as tile
from concourse import bacc
from concourse.bass_utils import run_bass_kernel_spmd
from concourse.tile import add_dep_helper
from concourse.masks import make_identity

# Problem shape (hardcoded; kernel.py must be self-contained).
N_NODES = 50000
F = 128
P = 3
E = 600000
N_CORES = 8
E_PC = E // N_CORES          # 75000 edges per core per power
HALF = 32768                 # int16 gather index limit

TILE = 512                   # edges per gather / per layer-1 matmul
GROUP = 4                    # layer-1 outputs packed per PSUM bank (offsets 0/32/64/96)
NQ = 4                       # SWDGE queues
# Per-(core,power) bucket capacities in tiles for buckets (src_half, tgt_half)
# = (0,0),(0,1),(1,0),(1,1). Expected sizes 32210/16940/16940/8909 edges;
# caps leave >8 sigma of headroom. Sum must be divisible by GROUP.
CAPS_TILES = (66, 35, 35, 20)

F16 = mybir.dt.float16
F32 = mybir.dt.float32
I16 = mybir.dt.int16


def build(caps=CAPS_TILES, n_cores=N_CORES):
    """Build + compile the SPMD Bass program. Returns (nc, meta)."""
    tiles_pp = sum(caps)
    assert tiles_pp % GROUP == 0
    slots_pp = tiles_pp * TILE
    side = P * slots_pp
    idxcols = side // 16

    nc = bacc.Bacc("TRN2", target_bir_lowering=False, debug=False,
                   num_devices=n_cores, num_swdge_queues=NQ)
    x = nc.dram_tensor("xf16", [N_NODES, F], F16, kind="ExternalInput")
    idxj = nc.dram_tensor("idxj", [128, idxcols], I16, kind="ExternalInput")
    idxi = nc.dram_tensor("idxi", [128, idxcols], I16, kind="ExternalInput")
    wa = nc.dram_tensor("wa", [F, 3 * P], F16, kind="ExternalInput")
    wb = nc.dram_tensor("wb", [99, 4 * P], F16, kind="ExternalInput")
    bat = nc.dram_tensor("bat", [99, P], F32, kind="ExternalInput")
    bbt = nc.dram_tensor("bbt", [4, P], F32, kind="ExternalInput")
    out = nc.dram_tensor("out", [P, tiles_pp, TILE], F32, kind="ExternalOutput")

    # Static bucket tile ranges within a power.
    bstart = [0]
    for c in caps:
        bstart.append(bstart[-1] + c)

    with tile.TileContext(nc) as tc:
        with (
            tc.tile_pool(name="const", bufs=1) as constp,
            tc.tile_pool(name="gj", bufs=12) as gjp,
            tc.tile_pool(name="gi", bufs=12) as gip,
            tc.tile_pool(name="dt", bufs=4) as dtp,
            tc.tile_pool(name="hr", bufs=4) as hrp,
            tc.tile_pool(name="ost", bufs=4) as ostp,
            tc.tile_pool(name="hp", bufs=4, space="PSUM") as hpp,
            tc.tile_pool(name="tp", bufs=2, space="PSUM") as tpp,
            tc.tile_pool(name="wp", bufs=2, space="PSUM") as wpp,
        ):
            idxj_sb = constp.tile([128, idxcols], I16)
            nc.sync.dma_start(idxj_sb[:], idxj.ap())
            idxi_sb = constp.tile([128, idxcols], I16)
            nc.sync.dma_start(idxi_sb[:], idxi.ap())
            wa_sb = constp.tile([F, 3 * P], F16)
            nc.sync.dma_start(wa_sb[:], wa.ap())
            wb_sb = constp.tile([99, 4 * P], F16)
            nc.sync.dma_start(wb_sb[:], wb.ap())
            bat_sb = constp.tile([99, P], F32)
            nc.sync.dma_start(bat_sb[:], bat.ap())
            bbt_sb = constp.tile([4, P], F32)
            nc.sync.dma_start(bbt_sb[:], bbt.ap())
            ident = constp.tile([128, 128], F16)
            make_identity(nc, ident[:])

            # Layer-1 PSUM banks: zero once so the partitions the matmuls
            # never write stay 0 (the block-diagonal layer-2 weights hit them
            # with 0s; 0*0 must not be NaN*0).
            hp_tiles = []
            for _ in range(4):
                t = hpp.tile([128, TILE], F32, tag="hp")
                nc.vector.memset(t[:], 0.0)
                hp_tiles.append(t)

            x_full = x.ap()
            x_high = x.ap()[HALF:, :]

            qn = 0
            prev_gather = None
            for p in range(P):
                wa_ap = wa_sb[:, 3 * p:3 * p + 3]
                wb_ap = wb_sb[:, 4 * p:4 * p + 4]
                ba_ap = bat_sb[:, p:p + 1]
                bb_ap = bbt_sb[:, p:p + 1]
                for t in range(tiles_pp):
                    b = next(bi for bi in range(4) if bstart[bi] <= t < bstart[bi + 1])
                    hj, hi = b >> 1, b & 1
                    col0 = (p * slots_pp + t * TILE) // 16
                    # Non-transpose gather: edge k of the tile lands on
                    # partition k%128, block k//128. (Concurrent TRANSPOSE
                    # gathers on different SWDGE queues corrupt data - shared
                    # xbar state - so we gather untransposed and transpose on
                    # the TensorEngine instead.)
                    gj = gjp.tile([128, TILE // 128, F], F16, tag="gj")
                    gi = gip.tile([128, TILE // 128, F], F16, tag="gi")
                    gj_inst = nc.gpsimd.dma_gather(
                        gj[:, :, :], x_high if hj else x_full,
                        idxj_sb[:, col0:col0 + TILE // 16],
                        num_idxs=TILE, num_idxs_reg=TILE, elem_size=F,
                        transpose=False, queue_num=qn % NQ)
                    qn += 1
                    # Chain gathers in emission order (no sem): keeps the Pool
                    # program order equal to emission order so Tile's DMASW
                    # sem-lane round-robin (8 lanes) stays in lockstep with
                    # the queue cycle (4 queues) - each sem lane then only
                    # ever serves one SWDGE queue, which the runtime requires.
                    if prev_gather is not None:
                        add_dep_helper(gj_inst.ins, prev_gather.ins,
                                       sync=False, reason="swdge lane lockstep")
                    gi_inst = nc.gpsimd.dma_gather(
                        gi[:, :, :], x_high if hi else x_full,
                        idxi_sb[:, col0:col0 + TILE // 16],
                        num_idxs=TILE, num_idxs_reg=TILE, elem_size=F,
                        transpose=False, queue_num=qn % NQ)
                    qn += 1
                    add_dep_helper(gi_inst.ins, gj_inst.ins,
                                   sync=False, reason="swdge lane lockstep")
                    prev_gather = gi_inst
                    dj = gj[:, :, :].rearrange("p a b -> p (a b)")
                    nc.vector.tensor_tensor(
                        dj, dj, gi[:, :, :].rearrange("p a b -> p (a b)"),
                        mybir.AluOpType.subtract)
                    # Transpose signed d blocks [128e,128f] -> [128f,128e] on
                    # PE; the |.| is folded into the PSUM -> SBUF copy as one
                    # ACT Abs activation (a DVE bitwise-abs measured 1.8us
                    # per tile - int ops get no 16-bit speedup - while ACT
                    # sits mostly idle here).
                    tp = tpp.tile([128, TILE // 128, 128], F16, tag="tp")
                    for blk in range(TILE // 128):
                        nc.tensor.transpose(tp[:, blk, :], gj[:, blk, :],
                                            ident[:])
                    dT = dtp.tile([128, TILE], F16, tag="dT")
                    tpf = tp[:, :, :].rearrange("p a b -> p (a b)")
                    nc.scalar.activation(dT[:, :], tpf[:, :],
                                         mybir.ActivationFunctionType.Abs)
                    g, q = divmod(t, GROUP)
                    hp = hp_tiles[g % 4]
                    nc.tensor.matmul(
                        hp[32 * q:32 * q + 3, :], lhsT=wa_ap, rhs=dT[:, :],
                        start=True, stop=True, tile_position=(0, 32 * q))
                    if q == GROUP - 1:
                        hr = hrp.tile([99, TILE], F16, tag="hr")
                        nc.scalar.activation(
                            hr[:], hp[:99, :],
                            mybir.ActivationFunctionType.Relu,
                            bias=ba_ap, scale=1.0)
                        wpt = wpp.tile([4, TILE], F32, tag="wp")
                        nc.tensor.matmul(wpt[:], lhsT=wb_ap, rhs=hr[:],
                                         start=True, stop=True)
                        ost = ostp.tile([4, TILE], F32, tag="ost")
                        nc.scalar.activation(
                            ost[:], wpt[:],
                            mybir.ActivationFunctionType.Sigmoid,
                            bias=bb_ap, scale=1.0)
                        nc.sync.dma_start(
                            out.ap()[p, GROUP * g:GROUP * g + GROUP, :],
                            ost[:])

    nc.compile()
    meta = {"tiles_pp": tiles_pp, "slots_pp": slots_pp, "caps": caps}
    return nc, meta


def prepare(inputs, caps=CAPS_TILES, n_cores=N_CORES, e_pc=E_PC):
    """Host-side sharding/bucketing. Returns (in_maps, slot_maps)."""
    x = np.asarray(inputs["x"])
    ei = np.asarray(inputs["edge_index"]).astype(np.int64)
    Wa = np.asarray(inputs["Wa"], dtype=np.float32)
    ba = np.asarray(inputs["ba"], dtype=np.float32)
    Wb = np.asarray(inputs["Wb"], dtype=np.float32)
    bb = np.asarray(inputs["bb"], dtype=np.float32)

    tiles_pp = sum(caps)
    slots_pp = tiles_pp * TILE
    caps_slots = np.asarray(caps, dtype=np.int64) * TILE
    starts = np.concatenate([[0], np.cumsum(caps_slots)[:-1]])

    xf16 = np.ascontiguousarray(x.astype(np.float16))
    wa_h = np.zeros((F, 3 * P), np.float16)
    wb_h = np.zeros((99, 4 * P), np.float16)
    bat_h = np.zeros((99, P), np.float32)
    bbt_h = np.zeros((4, P), np.float32)
    for p in range(P):
        wa_h[:, 3 * p:3 * p + 3] = Wa[p]
        bbt_h[:, p] = bb[p, 0]
        for q in range(GROUP):
            for k in range(3):
                wb_h[32 * q + k, 4 * p + q] = Wb[p, k, 0]
                bat_h[32 * q + k, p] = ba[p, k]

    def wrap(a):
        return np.tile(np.ascontiguousarray(a.reshape(-1, 16).T), (8, 1))

    in_maps, slot_maps = [], []
    for c in range(n_cores):
        idxj_all = np.zeros(P * slots_pp, np.int16)
        idxi_all = np.zeros(P * slots_pp, np.int16)
        slot_map = np.zeros((P, e_pc), np.int64)
        for p in range(P):
            j = ei[p, 0, c * e_pc:(c + 1) * e_pc]
            i = ei[p, 1, c * e_pc:(c + 1) * e_pc]
            hj = (j >= HALF).astype(np.int64)
            hi = (i >= HALF).astype(np.int64)
            bkt = hj * 2 + hi
            counts = np.bincount(bkt, minlength=4)
            if np.any(counts > caps_slots):
                raise RuntimeError(f"bucket overflow: {counts} vs {caps_slots}")
            order = np.argsort(bkt, kind="stable")
            slot_sorted = np.concatenate(
                [starts[b] + np.arange(counts[b]) for b in range(4)])
            slot = np.empty(e_pc, np.int64)
            slot[order] = slot_sorted
            base = p * slots_pp
            idxj_all[base + slot] = (j - HALF * hj).astype(np.int16)
            idxi_all[base + slot] = (i - HALF * hi).astype(np.int16)
            slot_map[p] = slot
        in_maps.append({
            "xf16": xf16,
            "idxj": wrap(idxj_all),
            "idxi": wrap(idxi_all),
            "wa": wa_h, "wb": wb_h, "bat": bat_h, "bbt": bbt_h,
        })
        slot_maps.append(slot_map)
    return in_maps, slot_maps


_CACHE = {}


def get_program():
    if "nc" not in _CACHE:
        _CACHE["nc"], _CACHE["meta"] = build()
    return _CACHE["nc"], _CACHE["meta"]


def postprocess(results, slot_maps, slots_pp, e_pc=E_PC, n_cores=N_CORES):
    out = np.empty((P, E), np.float32)
    for c in range(n_cores):
        w = results[c]["out"].reshape(P, slots_pp)
        out[:, c * e_pc:(c + 1) * e_pc] = np.take_along_axis(
            w, slot_maps[c], axis=1)
    return out


def kernel(**inputs) -> np.ndarray:
    nc, meta = get_program()
    in_maps, slot_maps = prepare(inputs)
    res = run_bass_kernel_spmd(nc, in_maps, core_ids=list(range(N_CORES)))
    return postprocess(res.results, slot_maps, meta["slots_pp"])
